# revision 3
# baseline (speedup 1.0000x reference)
"""DLinear forecast model as a single fused matmul on 8 TRN2 NeuronCores.

The model is out[b,p,c] = relu( sum_t seasonal[b,t,c]*Ws[p,t] + bs[p]
                               + sum_t trend[b,t,c]*Wt[p,t]    + bt[p] )
with trend = moving_avg(x) (kernel 5, edge pad) and seasonal = x - trend.
The moving average is a linear map over the time axis: trend = A @ x with
A [336,336].  Folding it into the weights gives a single matmul:
    out = relu(W_eff @ x[b] + bias),  W_eff = Ws + (Wt - Ws) @ A
Sharding: data-parallel over batch (64 = 8 cores x 8).

Orientation: x-stationary.  lhsT = x chunks [112t x 128c] (128-column
stationary tiles keep the fast-weight-load path), moving = W [112t, 720p].
PSUM tiles are [cw, 720] f32 (2 banks; each matmul writes within one bank:
512 + 208).  PE streaming cost is 14 c-chunks x 3 k-chunks x 720 = 30,240
cycles/batch vs 32,076 for the p-stationary layout (6 m-chunks x 1782).
Device output is [b, c, p]; the host transposes and applies bias + relu
(f32, exact).
"""

import numpy as np
import ml_dtypes

import concourse.bass as bass
import concourse.mybir as mybir
from concourse.tile import TileContext
from concourse.bass_utils import run_bass_kernel_spmd

# Problem shapes (hardcoded per contract)
B, T, C = 64, 336, 1782
P_OUT = 720
N_CORES = 8
B_LOC = B // N_CORES  # 8 batches per core

KCH = 112   # contraction chunk (3 chunks of 112 = 336)
NK = 3
NCH = 512   # moving-dim chunk: 720 = 512 + 208, each within one PSUM bank

BF16 = mybir.dt.bfloat16
F32 = mybir.dt.float32

C_SPLITS = [(i * 128, min(128, C - i * 128)) for i in range((C + 127) // 128)]
N_SPLITS = [(0, NCH), (NCH, P_OUT - NCH)]


def _split_excess_waits(nc, limit=1):
    """walrus in this toolchain rejects >limit sem-waits per instruction; move
    the extras onto injected same-engine NoOps immediately before it (same
    engine queue => program order => semantics preserved)."""
    seq = 0
    for f in nc.m.functions:
        for bb in f.blocks:
            new = []
            for inst in bb.instructions:
                si = inst.sync_info
                if si is not None and si.on_wait and len(si.on_wait) > limit:
                    waits = list(si.on_wait)
                    head, tail = waits[:-limit], waits[-limit:]
                    for w in head:
                        seq += 1
                        nop = mybir.InstNoOp(
                            name=f"{inst.name}-prewait{seq}", engine=inst.engine
                        )
                        nop.sync_info = mybir.SyncInfo(on_wait=[w], on_update=[])
                        new.append(nop)
                    inst.sync_info = mybir.SyncInfo(on_wait=tail, on_update=si.on_update)
                new.append(inst)
            bb.instructions = new


def build_kernel(reps=1, store_split=False):
    nc = bass.Bass()
    x = nc.declare_dram_parameter("x", [B_LOC, T, C], BF16, isOutput=False)
    w = nc.declare_dram_parameter("w", [T, P_OUT], BF16, isOutput=False)
    out = nc.declare_dram_parameter("out", [B_LOC, C, P_OUT], BF16, isOutput=True)

    with TileContext(nc) as tc:
        with (
            tc.tile_pool(name="wpool", bufs=1) as wpool,
            tc.tile_pool(name="xpool", bufs=6) as xpool,
            tc.tile_pool(name="opool", bufs=8) as opool,
            tc.tile_pool(name="psum", bufs=4, space="PSUM") as pspool,
        ):
            # x loads on the SP HW-DGE ring, w loads on ACT: descriptor
            # generation for both runs in parallel during the ramp.
            def load_x(b, k):
                t = xpool.tile([KCH, C], BF16, tag=f"x{k}")
                nc.sync.dma_start(out=t[:], in_=x[b, k * KCH : (k + 1) * KCH, :])
                return t

            xt_next = [load_x(0, 0)]
            wt = []
            for k in range(NK):
                t = wpool.tile([KCH, P_OUT], BF16, tag=f"w{k}")
                nc.scalar.dma_start(out=t[:], in_=w[k * KCH : (k + 1) * KCH, :])
                wt.append(t)
            xt_next += [load_x(0, 1), load_x(0, 2)]

            # PE warmup: dummy matmuls on memset tiles fill the initial
            # DMA wait and lift the HAM clock gate before the real work.
            dz = wpool.tile([KCH, NCH], BF16, tag="warm")
            nc.vector.memset(dz[:], 0.0)
            psw = pspool.tile([128, P_OUT], F32, tag="ps")
            for _ in range(8):
                nc.tensor.matmul(
                    psw[:KCH, :NCH], dz[:, :KCH], dz[:, :NCH], start=True, stop=True
                )

            for i in range(B_LOC * reps):
                b = i % B_LOC
                xt = xt_next
                xt_next = []
                for ci, (coff, cw) in enumerate(C_SPLITS):
                    ot = opool.tile([128, P_OUT], BF16, tag="o")
                    ps = pspool.tile([128, P_OUT], F32, tag="ps")
                    for noff, nw in N_SPLITS:
                        for k in range(NK):
                            nc.tensor.matmul(
                                ps[:cw, noff : noff + nw],
                                xt[k][:, coff : coff + cw],
                                wt[k][:, noff : noff + nw],
                                start=(k == 0), stop=(k == NK - 1),
                            )
                    # evict psum -> sbuf bf16 split across DVE and ACT
                    nc.vector.tensor_scalar_add(ot[:cw, :NCH], ps[:cw, :NCH], 0.0)
                    nc.scalar.copy(ot[:cw, NCH:], ps[:cw, NCH:])
                    store_eng = nc.scalar if (store_split and ci % 2) else nc.sync
                    store_eng.dma_start(
                        out=out[b, coff : coff + cw, :], in_=ot[:cw, :]
                    )
                    if ci < NK and i + 1 < B_LOC * reps:
                        xt_next.append(load_x((i + 1) % B_LOC, ci))

    _split_excess_waits(nc)
    return nc


def host_weights(W_seasonal, b_seasonal, W_trend, b_trend):
    """Fold the moving average into one weight matrix (f64 precision)."""
    K, PAD = 5, 2
    A = np.zeros((T, T), dtype=np.float64)
    idx = np.arange(T)
    for d in range(-PAD, PAD + 1):
        np.add.at(A, (idx, np.clip(idx + d, 0, T - 1)), 1.0 / K)
    Ws = W_seasonal.astype(np.float64)
    Wt = W_trend.astype(np.float64)
    W_eff = Ws + (Wt - Ws) @ A  # [720, 336]
    bias = (b_seasonal.astype(np.float64) + b_trend.astype(np.float64)).astype(
        np.float32
    )
    wT = np.ascontiguousarray(W_eff.T.astype(np.float32)).astype(ml_dtypes.bfloat16)
    return wT, bias


def make_in_maps(x, W_seasonal, b_seasonal, W_trend, b_trend):
    wT, _ = host_weights(W_seasonal, b_seasonal, W_trend, b_trend)
    xb = np.asarray(x).astype(ml_dtypes.bfloat16)
    return [
        {
            "x": np.ascontiguousarray(xb[i * B_LOC : (i + 1) * B_LOC]),
            "w": wT,
        }
        for i in range(N_CORES)
    ]


def kernel(x, W_seasonal, b_seasonal, W_trend, b_trend):
    x = np.asarray(x)
    W_seasonal = np.asarray(W_seasonal)
    b_seasonal = np.asarray(b_seasonal)
    W_trend = np.asarray(W_trend)
    b_trend = np.asarray(b_trend)
    in_maps = make_in_maps(x, W_seasonal, b_seasonal, W_trend, b_trend)
    _, bias = host_weights(W_seasonal, b_seasonal, W_trend, b_trend)
    for attempt in range(3):
        try:
            nc = build_kernel()
            res = run_bass_kernel_spmd(nc, in_maps, core_ids=list(range(N_CORES)))
            break
        except Exception:  # transient device wedge (NRT_EXEC_UNIT_...)
            if attempt == 2:
                raise
            import time as _time

            _time.sleep(20)
    parts = [res.results[i]["out"].astype(np.float32) for i in range(N_CORES)]
    y = np.concatenate(parts, axis=0)  # [B, C, P]
    y = np.ascontiguousarray(y.transpose(0, 2, 1))  # [B, P, C]
    y += bias[None, :, None]
    np.maximum(y, 0.0, out=y)
    return y
